# revision 1
# baseline (speedup 1.0000x reference)
"""DenseGCN (multi-edge-type) Trainium2 kernel.

Data-parallel over batch across 8 NeuronCores (8 graphs per core).

Math (per graph):
  adj_sl = adj with diagonal set to 1 (self loops), per edge type f
  deg[i,f] = clip(sum_j adj_sl[i,j,f], 1)^-0.5
  layer(h) = silu((sum_f D_f adj_sl_f D_f) @ (h W) + b) * mask
  Collapse edge types:  A2[i,j] = sum_f deg[i,f]*adj[i,j,f]*deg[j,f]
  self-loop correction as a diagonal add: A2full = A2 + diag(Cs),
  Cs[i] = sum_f deg[i,f]^2 * (1 - adj[i,i,f]).

Structure per graph (2-stage software pipeline across graphs):
  stage_load:  adj DMA -> deg row-sums (2 chunks ACT accum, 2 chunks DVE
               strided reduce) -> rsqrt via DVE Newton (keeps ACT on the
               silu table only, no act-table reloads) -> deg_i row scale
               (DVE broadcast multiply) -> x transpose + x@W0 on PE.
  stage_comp:  PE 128x128 fp32 block transposes -> DVE fused deg_j-scale
               + edge-type-sum into A2T [j part, i free] (+ Cs diagonal
               block) -> layer 1 fully in transposed layout: psL1[H,i] =
               sum_cj h0^T A2T with bias as a rank-1 f32r matmul, ACT
               Silu -> h1T; hw1 = h1T W1 with mask folded into the
               PSUM->SBUF copy as a per-partition ACT scale; layer 2
               likewise, then transpose back, ACT Silu -> h2 natural,
               masked mean pool via matmul. MLP head batched across all
               graphs at the end (one matmul chain instead of 8).
"""

import os

import numpy as np

import concourse.bass as bass
from concourse import bacc, masks, mybir, tile
from concourse.bass_utils import run_bass_kernel_spmd

B, N, F = 64, 512, 4
IN, H, OUT = 64, 128, 16
NCORES = 8
BPC = B // NCORES  # graphs per core
P = 128
NCH = N // P  # 4 chunks of 128 nodes

dt = mybir.dt
AF = mybir.ActivationFunctionType
ALU = mybir.AluOpType
AXL = mybir.AxisListType

MAGIC = 0x5F3759DF
# load adj as bf16 (DMA converts): DVE 2x rowscale, 1 cyc/row transposes
BF16_ADJ = os.environ.get("GCN_BF16_ADJ", "1") == "1"
ADJDT = dt.bfloat16 if BF16_ADJ else dt.float32


def build_nc(n_batches=BPC):
    nc = bacc.Bacc(
        "TRN2", target_bir_lowering=False, debug=False, enable_asserts=False
    )

    adj_d = nc.dram_tensor(
        "adj", [n_batches, N, N, F], dt.float32, kind="ExternalInput"
    )
    x_d = nc.dram_tensor("x", [n_batches, N, IN], dt.float32, kind="ExternalInput")
    mask_d = nc.dram_tensor("mask", [n_batches, N], dt.int32, kind="ExternalInput")
    W0_d = nc.dram_tensor("W0", [IN, H], dt.float32, kind="ExternalInput")
    b0_d = nc.dram_tensor("b0", [H], dt.float32, kind="ExternalInput")
    W1_d = nc.dram_tensor("W1", [H, H], dt.float32, kind="ExternalInput")
    b1_d = nc.dram_tensor("b1", [H], dt.float32, kind="ExternalInput")
    Wl1_d = nc.dram_tensor("Wl1", [H, H], dt.float32, kind="ExternalInput")
    bl1_d = nc.dram_tensor("bl1", [H], dt.float32, kind="ExternalInput")
    Wl2_d = nc.dram_tensor("Wl2", [H, OUT], dt.float32, kind="ExternalInput")
    bl2_d = nc.dram_tensor("bl2", [OUT], dt.float32, kind="ExternalInput")
    out_d = nc.dram_tensor("out", [n_batches, OUT], dt.float32, kind="ExternalOutput")

    f32r = dt.float32r

    with tile.TileContext(nc) as tc:
        with (
            tc.tile_pool(name="const", bufs=1) as constp,
            tc.tile_pool(name="adjp", bufs=2 * NCH) as adjp,
            tc.tile_pool(name="a2p", bufs=3 * NCH) as a2p,
            tc.tile_pool(name="hp", bufs=3) as hp,
            tc.tile_pool(name="smallp", bufs=3) as smallp,
            tc.tile_pool(name="medp", bufs=2) as medp,
            tc.tile_pool(name="psA", bufs=2, space="PSUM") as psA,
            tc.tile_pool(name="psB", bufs=3, space="PSUM") as psB,
            tc.tile_pool(name="psC", bufs=1, space="PSUM") as psC,
            tc.tile_pool(name="psD", bufs=1, space="PSUM") as psD,
        ):
            identF = constp.tile([P, P], dt.float32)
            masks.make_identity(nc, identF[:])
            identB = constp.tile([P, P], ADJDT)
            nc.vector.tensor_copy(identB[:], identF[:])

            W0s = constp.tile([IN, H], dt.float32)
            nc.sync.dma_start(W0s[:], W0_d.ap())
            W1s = constp.tile([H, H], dt.float32)
            nc.sync.dma_start(W1s[:], W1_d.ap())
            Wl1s = constp.tile([H, H], dt.float32)
            nc.sync.dma_start(Wl1s[:], Wl1_d.ap())
            Wl2s = constp.tile([H, OUT], dt.float32)
            nc.sync.dma_start(Wl2s[:], Wl2_d.ap())
            # biases as [H, 1] columns: per-partition ACT bias in the
            # transposed [H, i] layout (exact fp32)
            b0col = constp.tile([H, 1], dt.float32)
            nc.sync.dma_start(b0col[:], b0_d.ap().rearrange("(p o) -> p o", o=1))
            b1col = constp.tile([H, 1], dt.float32)
            nc.sync.dma_start(b1col[:], b1_d.ap().rearrange("(p o) -> p o", o=1))
            bl1c = constp.tile([H, 1], dt.float32)
            nc.sync.dma_start(bl1c[:], bl1_d.ap().rearrange("(p o) -> p o", o=1))
            bl2c = constp.tile([OUT, 1], dt.float32)
            nc.sync.dma_start(bl2c[:], bl2_d.ap().rearrange("(p o) -> p o", o=1))
            # pooled per-graph embeddings, collected as one [1, B*H] row
            gAll = constp.tile([1, n_batches * H], dt.float32)
            outS = constp.tile([OUT, n_batches], dt.float32)

            def dma_issue(b):
                st = {}
                adjN = []
                for ci in range(NCH):
                    t = adjp.tile([P, N, F], dt.float32, tag="adjN")
                    nc.sync.dma_start(t[:], adj_d.ap()[b, ci * P : (ci + 1) * P])
                    adjN.append(t)
                st["adjN"] = adjN
                diagN = smallp.tile([P, NCH, F], dt.float32, tag="diag")
                nc.sync.dma_start(
                    diagN[:],
                    bass.AP(
                        tensor=adj_d,
                        offset=b * N * N * F,
                        ap=[[(N * F + F), P], [(N * F + F) * P, NCH], [1, F]],
                    ),
                )
                xb = smallp.tile([P, NCH, IN], dt.float32, tag="xb")
                nc.sync.dma_start(
                    xb[:], x_d.ap()[b].rearrange("(c p) d -> p c d", p=P)
                )
                mi = smallp.tile([P, NCH], dt.int32, tag="mi")
                nc.sync.dma_start(
                    mi[:], mask_d.ap()[b].rearrange("(c p) -> p c", p=P)
                )
                st["diagN"] = diagN
                st["xb"] = xb
                st["mi"] = mi
                return st

            def load_math(b, st):
                adjN = st["adjN"]
                diagN = st["diagN"]
                xb = st["xb"]
                mi = st["mi"]
                maskb = smallp.tile([P, NCH], dt.float32, tag="maskb")
                nc.vector.tensor_copy(maskb[:], mi[:])
                st["maskb"] = maskb

                # degrees: row sums. 2 chunks ScalarE accumulate, 2 DVE.
                degsum = smallp.tile([P, NCH, F], dt.float32, tag="degsum")
                junk = medp.tile([P, N], dt.float32, tag="junk")
                for ci in range(NCH):
                    if ci < 3:
                        for f in range(F):
                            nc.scalar.activation(
                                junk[:],
                                adjN[ci][:, :, f],
                                AF.Copy,
                                accum_out=degsum[:, ci, f : f + 1],
                            )
                    else:
                        nc.vector.tensor_reduce(
                            degsum[:, ci, :],
                            adjN[ci][:].transpose([0, 2, 1]),
                            axis=AXL.X,
                            op=ALU.add,
                        )
                # dtmp = max(degsum + 1 - diag, 1)
                dtmp = smallp.tile([P, NCH, F], dt.float32, tag="dtmp")
                nc.vector.tensor_tensor(dtmp[:], degsum[:], diagN[:], ALU.subtract)
                nc.vector.tensor_scalar(dtmp[:], dtmp[:], 1.0, 1.0, ALU.add, ALU.max)
                # deg = dtmp^-0.5: fast-inverse-sqrt bit trick + 2 Newton
                # steps, all on DVE (keeps ACT on a single act table)
                ti = smallp.tile([P, NCH, F], dt.int32, tag="ti")
                nc.vector.tensor_scalar(
                    ti[:], dtmp[:].bitcast(dt.int32), 1, None, ALU.arith_shift_right
                )
                nc.vector.tensor_scalar(ti[:], ti[:], -1, MAGIC, ALU.mult, ALU.add)
                e = smallp.tile([P, NCH, F], dt.float32, tag="nwt")
                deg = ti[:].bitcast(dt.float32)
                for _ in range(3):
                    nc.vector.tensor_tensor(e[:], dtmp[:], deg, ALU.mult)
                    nc.vector.tensor_tensor(e[:], e[:], deg, ALU.mult)
                    nc.vector.tensor_scalar(
                        e[:], e[:], -0.5, 1.5, ALU.mult, ALU.add
                    )
                    nc.vector.tensor_tensor(deg, deg, e[:], ALU.mult)
                st["deg"] = deg
                # row scale adj by deg_i; writes a bf16 copy (RNE) that the
                # PE transposes consume at 1 cyc/row
                adjS = []
                for ci in range(NCH):
                    s = adjp.tile([P, N, F], ADJDT, tag="adjS")
                    nc.vector.tensor_tensor(
                        s[:],
                        adjN[ci][:, :, :],
                        deg[:, ci, None, :].to_broadcast([P, N, F]),
                        ALU.mult,
                    )
                    adjS.append(s)
                st["adjS"] = adjS
                # Cs = sum_f deg^2 * (1 - diag)
                om = smallp.tile([P, NCH, F], dt.float32, tag="om")
                nc.vector.tensor_scalar(
                    om[:], diagN[:], -1.0, 1.0, ALU.mult, ALU.add
                )
                csf = smallp.tile([P, NCH, F], dt.float32, tag="csf")
                nc.vector.tensor_tensor(csf[:], deg[:], deg[:], ALU.mult)
                nc.vector.tensor_tensor(csf[:], csf[:], om[:], ALU.mult)
                Cs = smallp.tile([P, NCH], dt.float32, tag="Cs")
                nc.vector.tensor_reduce(Cs[:], csf[:], axis=AXL.X, op=ALU.add)
                st["Cs"] = Cs
                maskdiv = smallp.tile([P, NCH], dt.float32, tag="md")
                nc.vector.tensor_scalar_mul(maskdiv[:], maskb[:], 1.0 / N)
                st["maskdiv"] = maskdiv

                # h0 = x @ W0 (natural [j, H] layout, exact fp32)
                psX = psC.tile([IN, N], dt.float32, tag="px")
                for ci in range(NCH):
                    nc.tensor.transpose(
                        psX[:, ci * P : (ci + 1) * P], xb[:, ci, :], identF[:]
                    )
                xTs = medp.tile([IN, N], dt.float32, tag="xTs")
                nc.scalar.copy(xTs[:], psX[:])
                psH0 = psC.tile([P, NCH, H], dt.float32, tag="px")
                for ci in range(NCH):
                    nc.tensor.matmul(
                        psH0[:, ci, :],
                        xTs[:, ci * P : (ci + 1) * P],
                        W0s[:],
                        start=True,
                        stop=True,
                    )
                h0 = hp.tile([P, NCH, H], f32r, tag="h0")
                nc.scalar.copy(h0[:], psH0[:])
                st["h0"] = h0

            def stage_compute(b, st):
                adjS = st["adjS"]
                deg = st["deg"]
                Cs = st["Cs"]
                maskb = st["maskb"]
                maskdiv = st["maskdiv"]

                # Cs diagonal blocks (per cj), built on DVE (cheap 128-free)
                csd = []
                for cj in range(NCH):
                    t = smallp.tile([P, P], dt.float32, tag="csd")
                    nc.scalar.activation(
                        t[:], identF[:], AF.Copy, scale=Cs[:, cj : cj + 1]
                    )
                    csd.append(t)

                # transpose + assemble A2T [j, i] (+ Cs diag)
                A2T = []
                for cj in range(NCH):
                    acc = a2p.tile([P, N], f32r, tag="A2T")
                    for f in range(F):
                        BT = psA.tile([P, N], ADJDT, tag="BT")
                        for ci in range(NCH):
                            nc.tensor.transpose(
                                BT[:, ci * P : (ci + 1) * P],
                                adjS[ci][:, cj * P : (cj + 1) * P, f],
                                identB[:],
                            )
                        if f == 0:
                            nc.scalar.activation(
                                acc[:],
                                BT[:],
                                AF.Copy,
                                scale=deg[:, cj, 0:1],
                            )
                        else:
                            nc.vector.scalar_tensor_tensor(
                                acc[:],
                                BT[:],
                                deg[:, cj, f : f + 1],
                                acc[:],
                                op0=ALU.mult,
                                op1=ALU.add,
                            )
                    # self-loop correction on the diagonal block
                    nc.vector.tensor_tensor(
                        acc[:, cj * P : (cj + 1) * P],
                        acc[:, cj * P : (cj + 1) * P],
                        csd[cj][:],
                        ALU.add,
                    )
                    A2T.append(acc)

                # two GCN layers, transposed [H, i] layout
                hw = st["h0"]
                for l in range(2):
                    psL = psB.tile([H, N], dt.float32, tag="mm")
                    for cj in range(NCH):
                        nc.tensor.matmul(
                            psL[:],
                            hw[:, cj, :],
                            A2T[cj][:],
                            start=(cj == 0),
                            stop=(cj == NCH - 1),
                        )
                    if l == 0:
                        sg1 = medp.tile([H, N], dt.float32, tag="sg1")
                        nc.scalar.activation(
                            sg1[:], psL[:], AF.Sigmoid, bias=b0col[:, 0:1]
                        )
                        h1T = hp.tile([H, N], dt.float32, tag="h1T")
                        nc.vector.scalar_tensor_tensor(
                            h1T[:],
                            psL[:],
                            b0col[:, 0:1],
                            sg1[:],
                            op0=ALU.add,
                            op1=ALU.mult,
                        )
                        psW = psB.tile([P, NCH, H], dt.float32, tag="mm")
                        for cj in range(NCH):
                            nc.tensor.matmul(
                                psW[:, cj, :],
                                h1T[:, cj * P : (cj + 1) * P],
                                W1s[:],
                                start=True,
                                stop=True,
                            )
                        hw = hp.tile([P, NCH, H], f32r, tag="hw")
                        for cj in range(NCH):
                            nc.scalar.activation(
                                hw[:, cj, :],
                                psW[:, cj, :],
                                AF.Copy,
                                scale=maskb[:, cj : cj + 1],
                            )
                    else:
                        L2s = medp.tile([H, N], dt.float32, tag="L2s")
                        nc.scalar.activation(
                            L2s[:], psL[:], AF.Identity, bias=b1col[:, 0:1]
                        )
                        psN = psB.tile([P, NCH, H], dt.float32, tag="mm")
                        for ci in range(NCH):
                            nc.tensor.transpose(
                                psN[:, ci, :],
                                L2s[:, ci * P : (ci + 1) * P],
                                identF[:],
                            )
                        sg2 = hp.tile([P, NCH, H], dt.float32, tag="sg2")
                        nc.scalar.activation(sg2[:], psN[:], AF.Sigmoid)
                        h2 = hp.tile([P, NCH, H], dt.float32, tag="h2")
                        nc.vector.tensor_tensor(
                            h2[:], psN[:], sg2[:], ALU.mult
                        )

                # masked mean pool; per-graph embedding -> column b of psGT
                psG = psD.tile([1, H], dt.float32, tag="head")
                for ci in range(NCH):
                    nc.tensor.matmul(
                        psG[:],
                        maskdiv[:, ci : ci + 1],
                        h2[:, ci, :],
                        start=(ci == 0),
                        stop=(ci == NCH - 1),
                    )
                nc.scalar.copy(gAll[:, b * H : (b + 1) * H], psG[:])

            prev = None
            for b in range(n_batches):
                st = dma_issue(b)
                load_math(b, st)
                if prev is not None:
                    stage_compute(b - 1, prev)
                prev = st
            stage_compute(n_batches - 1, prev)

            # batched MLP head over all graphs: [H, B] columns
            psGT = psD.tile([H, n_batches], dt.float32, tag="head2")
            for b in range(n_batches):
                nc.tensor.transpose(
                    psGT[:, b : b + 1],
                    gAll[:, b * H : (b + 1) * H],
                    identF[0:1, 0:1],
                )
            gT = constp.tile([H, n_batches], dt.float32)
            nc.scalar.copy(gT[:], psGT[:])
            psH1 = psD.tile([H, n_batches], dt.float32, tag="head2")
            nc.tensor.matmul(psH1[:], Wl1s[:], gT[:], start=True, stop=True)
            g1s = constp.tile([H, n_batches], dt.float32)
            nc.scalar.activation(g1s[:], psH1[:], AF.Sigmoid, bias=bl1c[:, 0:1])
            g1 = constp.tile([H, n_batches], dt.float32)
            nc.vector.scalar_tensor_tensor(
                g1[:],
                psH1[:],
                bl1c[:, 0:1],
                g1s[:],
                op0=ALU.add,
                op1=ALU.mult,
            )
            psO = psD.tile([OUT, n_batches], dt.float32, tag="head2")
            nc.tensor.matmul(psO[:], Wl2s[:], g1[:], start=True, stop=True)
            nc.scalar.activation(outS[:], psO[:], AF.Identity, bias=bl2c[:, 0:1])

            nc.sync.dma_start(out_d.ap().rearrange("b c -> c b"), outS[:])

    nc.compile()
    return nc


_NC_CACHE = {}


def _get_nc(n_batches=BPC):
    if n_batches not in _NC_CACHE:
        _NC_CACHE[n_batches] = build_nc(n_batches)
    return _NC_CACHE[n_batches]


def make_in_maps(x, adj, mask, W0, b0, W1, b1, Wl1, bl1, Wl2, bl2):
    ws = dict(
        W0=np.ascontiguousarray(W0, np.float32),
        b0=np.ascontiguousarray(b0, np.float32),
        W1=np.ascontiguousarray(W1, np.float32),
        b1=np.ascontiguousarray(b1, np.float32),
        Wl1=np.ascontiguousarray(Wl1, np.float32),
        bl1=np.ascontiguousarray(bl1, np.float32),
        Wl2=np.ascontiguousarray(Wl2, np.float32),
        bl2=np.ascontiguousarray(bl2, np.float32),
    )
    in_maps = []
    for c in range(NCORES):
        sl = slice(c * BPC, (c + 1) * BPC)
        m = dict(
            adj=np.ascontiguousarray(adj[sl], np.float32),
            x=np.ascontiguousarray(x[sl], np.float32),
            mask=np.ascontiguousarray(mask[sl], np.int32),
        )
        m.update(ws)
        in_maps.append(m)
    return in_maps


def kernel(x, adj, mask, W0, b0, W1, b1, Wl1, bl1, Wl2, bl2, **kw):
    nc = _get_nc()
    in_maps = make_in_maps(x, adj, mask, W0, b0, W1, b1, Wl1, bl1, Wl2, bl2)
    res = run_bass_kernel_spmd(nc, in_maps, core_ids=list(range(NCORES)))
    out = np.concatenate([res.results[c]["out"] for c in range(NCORES)], axis=0)
    return out.astype(np.float32)

